# revision 1
# baseline (speedup 1.0000x reference)
"""Trainium2 Bass kernel for cosine-similarity ("sparse") attention.

Reference computation (B=2, C=512, N=2048, H=16, D=64, SCALE=8):
    qkv = Wqkv @ x                          # 1x1 conv
    q,k,v -> [B,H,D,N]
    q = l2norm(q, over D) * q_scale ; k = l2norm(k, over D) * k_scale
    sim = (q^T k) * 8 ; attn = softmax(sim, over keys)
    out = Wout @ (attn @ v) + bout

Sharding: 32 (batch, head) pairs across 8 cores -> each core owns one batch
(b = core//4) and 4 heads (h0 = 4*(core%4)).  Each core projects q/k/v for
its heads, runs attention, and computes a partial output projection
Wout[:, its-heads] @ y + bout/4.  Host sums the 4 partials per batch.

Device-side layout choices (per core):
  - qT/kT kept as [D, N] ("transposed") so sim^T tiles [j, q] come straight
    off the PE: simT = kT_j.T @ qT.  exp() is applied to simT, and
    attn @ v becomes oT[d, q] = v_ext[j, d].T @ attnT[j, q] accumulated over
    j-chunks, where v_ext carries an extra ones-column producing the softmax
    denominator for free.  Softmax denominator division is deferred to the
    [65, 512] oT tiles (cheap) instead of the [2048, 2048] attn matrix.
  - Softmax max-subtraction is skipped: sim = 8*cosine is bounded in [-8, 8].
  - All matmuls run as float32r (full PE rate at free-dim >= 256).
  - Head-pair packing: 2 heads per 128-partition tile; the K=64 sim matmuls
    for the two heads use PE row-groups (tile_position (0,0)/(64,0)) and run
    concurrently in the array.
"""

import os
import sys

import numpy as np

sys.path.insert(0, "/opt/trn_rl_repo")

import concourse.bass as bass  # noqa: E402
import concourse.mybir as mybir  # noqa: E402
from concourse import bacc, tile  # noqa: E402
from concourse.bass_utils import run_bass_kernel_spmd  # noqa: E402

F32 = mybir.dt.float32
F32R = mybir.dt.float32r
BF16 = mybir.dt.bfloat16
AF = mybir.ActivationFunctionType
OP = mybir.AluOpType

B, C, N = 2, 512, 2048
HEADS, D = 16, 64
SCALE = 8.0
NCORES = 8
HPC = 4  # heads per core

_CACHED_NC = None
LAST_RESULTS = None
EXTRA_RUN_KWARGS = {}


def build_nc(dbg=False):
    nc = bacc.Bacc(None, target_bir_lowering=False)

    x_d = nc.declare_dram_parameter("x", [C, N], F32R, isOutput=False)
    wqT_d = nc.declare_dram_parameter("wqT", [C, HPC * D], F32R, isOutput=False)
    wkT_d = nc.declare_dram_parameter("wkT", [C, HPC * D], F32R, isOutput=False)
    wvT_d = nc.declare_dram_parameter("wvT", [C, HPC * D], F32R, isOutput=False)
    woT_d = nc.declare_dram_parameter("woT", [HPC * D, C], F32R, isOutput=False)
    qsks8_d = nc.declare_dram_parameter("qsks8", [128, 1], F32, isOutput=False)
    onesw_d = nc.declare_dram_parameter("onesw", [128, 33], F32R, isOutput=False)
    vones_d = nc.declare_dram_parameter("vones", [128, 64], BF16, isOutput=False)
    biasq_d = nc.declare_dram_parameter("biasq", [C, 1], F32, isOutput=False)
    out_d = nc.declare_dram_parameter("out", [C, N], F32, isOutput=True)
    if dbg:
        dbg_qn_d = nc.declare_dram_parameter("dbg_qn", [256, N], F32R, isOutput=True)
        dbg_kn_d = nc.declare_dram_parameter("dbg_kn", [256, N], F32R, isOutput=True)
        dbg_inv_d = nc.declare_dram_parameter("dbg_inv", [256, N], F32, isOutput=True)
        dbg_vext_d = nc.declare_dram_parameter(
            "dbg_vext", [128, 16 * HPC * (D + 1)], BF16, isOutput=True
        )
        dbg_y_d = nc.declare_dram_parameter("dbg_y", [256, N], F32R, isOutput=True)
        dbg_at_d = nc.declare_dram_parameter("dbg_at", [128, 1024], BF16, isOutput=True)

    NQT = N // 512  # 4 query chunks of 512
    NJ = N // 128  # 16 key chunks of 128
    NCT = C // 128  # 4 channel chunks of 128

    with tile.TileContext(nc) as tc:
        with (
            tc.tile_pool(name="const", bufs=1) as const,
            tc.tile_pool(name="persist", bufs=1) as persist,
            tc.tile_pool(name="dramp", bufs=1, space="DRAM") as dramp,
        ):
            qsks8_sb = const.tile([128, 1], F32, name="qsks8", tag="qsks8")
            nc.sync.dma_start(qsks8_sb[:], qsks8_d[:])
            biasq_sb = const.tile([128, NCT], F32, name="biasq", tag="biasq")
            for ct in range(NCT):
                nc.sync.dma_start(
                    biasq_sb[:, ct : ct + 1], biasq_d[ct * 128 : (ct + 1) * 128, :]
                )
            # indicator weights: col 0 sums partitions 0-63 (head A), col 32
            # sums partitions 64-127 (head B); middle cols write zeros so the
            # [33, 512] sumsq psum rows land 32-aligned.  (DMA'd from host:
            # engines cannot memset float32r tiles.)
            ones_w = const.tile([128, 33], F32R, name="ones_w", tag="ones_w")
            nc.sync.dma_start(ones_w[:], onesw_d[:])

            # persistent tensors
            qn = [persist.tile([128, N], F32R, name=f"qn{m}", tag=f"qn{m}") for m in range(2)]
            kn = [persist.tile([128, N], F32R, name=f"kn{m}", tag=f"kn{m}") for m in range(2)]
            y = [
                [
                    persist.tile([128, 512], F32R, name=f"y{m}_{qt}", tag=f"y{m}_{qt}")
                    for qt in range(4)
                ]
                for m in range(2)
            ]
            vext = persist.tile([128, NJ, HPC, D + 1], BF16, name="vext", tag="vext")
            inv_dram = dramp.tile([8, N], F32, name="inv_dram", tag="inv_dram")

            with (
                tc.tile_pool(name="xw", bufs=1) as xw,
                tc.tile_pool(name="raw", bufs=1) as rawp,
                tc.tile_pool(name="sq", bufs=3) as sqp,
                tc.tile_pool(name="bb", bufs=2) as bbp,
                tc.tile_pool(name="prps", bufs=4, space="PSUM") as prps,
                tc.tile_pool(name="ssps", bufs=2, space="PSUM") as ssps,
            ):
                # per-(q|k, pair) sumsq tiles, head A at row 0 / head B at
                # row 32 (base-0 so the custom-DVE reciprocal works on HW);
                # after recip the same tile holds the inverse norms
                sumsq_tm = [
                    [
                        rawp.tile([33, N], F32, name=f"sumsq{t}{m}", tag=f"sumsq{t}{m}")
                        for m in range(2)
                    ]
                    for t in range(2)
                ]
                srt_s = [
                    rawp.tile([33, N], F32, name=f"srt{m}", tag=f"srt{m}")
                    for m in range(2)
                ]
                # DMA order tuned for earliest first matmul: wq, x[nt0], wk,
                # x[nt1..3], wv; triggers spread over the three DMA-capable
                # engines' queues
                wq_all = xw.tile([128, NCT, HPC * D], F32R, name="wq_all", tag="wq_all")
                nc.scalar.dma_start(
                    wq_all[:], wqT_d[:].rearrange("(c p) d -> p c d", p=128)
                )
                wq_sb = [wq_all[:, c, :] for c in range(NCT)]
                dma_engs = [nc.sync, nc.scalar, nc.gpsimd, nc.sync]
                x_sb = [[None] * NQT for _ in range(NCT)]
                for c in range(NCT):
                    t = xw.tile([128, 512], F32R, name=f"x{c}_0", tag=f"x{c}_0")
                    dma_engs[c].dma_start(t[:], x_d[c * 128 : (c + 1) * 128, 0:512])
                    x_sb[c][0] = t
                wk_all = xw.tile([128, NCT, HPC * D], F32R, name="wk_all", tag="wk_all")
                nc.gpsimd.dma_start(
                    wk_all[:], wkT_d[:].rearrange("(c p) d -> p c d", p=128)
                )
                wk_sb = [wk_all[:, c, :] for c in range(NCT)]
                wv_all = xw.tile([128, NCT, HPC * D], F32R, name="wv_all", tag="wv_all")
                nc.scalar.dma_start(
                    wv_all[:], wvT_d[:].rearrange("(c p) d -> p c d", p=128)
                )
                wv_sb = [wv_all[:, c, :] for c in range(NCT)]
                for nt in range(1, NQT):
                    for c in range(NCT):
                        t = xw.tile([128, 512], F32R, name=f"x{c}_{nt}", tag=f"x{c}_{nt}")
                        dma_engs[c].dma_start(
                            t[:], x_d[c * 128 : (c + 1) * 128, nt * 512 : (nt + 1) * 512]
                        )
                        x_sb[c][nt] = t

                def proj_chunk(m, w_sb, raws, ti, nt):
                    ps = prps.tile([128, 512], F32, name="pr", tag="pr")
                    for c in range(NCT):
                        nc.tensor.matmul(
                            ps[:],
                            lhsT=(w_sb[c][:, m * 128 : (m + 1) * 128]),
                            rhs=(x_sb[c][nt][:]),
                            start=(c == 0),
                            stop=(c == NCT - 1),
                        )
                    nc.vector.tensor_copy(
                        raws[m][:, nt * 512 : (nt + 1) * 512], ps[:]
                    )
                    sq = sqp.tile([128, 512], F32R, name="sq", tag="sq")
                    nc.vector.tensor_tensor(
                        sq[:],
                        raws[m][:, nt * 512 : (nt + 1) * 512],
                        ps[:],
                        OP.mult,
                    )
                    ss = ssps.tile([33, 512], F32, name="ss", tag="ss")
                    nc.tensor.matmul(
                        ss[:], lhsT=(ones_w[:]), rhs=(sq[:]), start=True, stop=True
                    )
                    nc.vector.tensor_copy(
                        sumsq_tm[ti][m][:, nt * 512 : (nt + 1) * 512], ss[:]
                    )

                def vproj_chunk(nm_):
                    psv = prps.tile(
                        [128, HPC * D], F32, name="prv", tag="prv", bufs=2
                    )
                    for c in range(NCT):
                        nc.tensor.matmul(
                            psv[:],
                            lhsT=(
                                x_sb[c][nm_ // 4][
                                    :, (nm_ % 4) * 128 : (nm_ % 4) * 128 + 128
                                ]
                            ),
                            rhs=(wv_sb[c][:]),
                            start=(c == 0),
                            stop=(c == NCT - 1),
                        )
                    nc.vector.tensor_copy(
                        vext[:, nm_, :, 0:D],
                        psv[:].rearrange("p (h d) -> p h d", h=HPC),
                    )

                # pair-0 q/k projections interleaved with the v projection,
                # nt-major, so PE density tracks the x DMA arrival; pair-1
                # afterwards overlaps pair-0's norm chain
                for m in range(2):
                    for nt in range(NQT):
                        proj_chunk(m, wq_sb, qn, 0, nt)
                        proj_chunk(m, wk_sb, kn, 1, nt)
                    # norm chain for pair m, overlaps the next pair's
                    # projection matmuls on the PE
                    for ti in range(2):
                        nc.scalar.activation(
                            srt_s[m][:], sumsq_tm[ti][m][:], AF.Sqrt
                        )
                        nc.vector.reciprocal_approx_fast(
                            sumsq_tm[ti][m][:], srt_s[m][:]
                        )
                        # rows {0, 32} -> inv_dram[4*ti + 2*m : +2] in one DMA
                        nc.sync.dma_start(
                            inv_dram[4 * ti + 2 * m : 4 * ti + 2 * m + 2, :],
                            sumsq_tm[ti][m][0:33:32, :],
                        )
                    # broadcast inverse norms and normalize qn/kn of pair m
                    bt_engs = [nc.scalar, nc.gpsimd]
                    for dsts, ti in ((qn, 0), (kn, 1)):
                        rowA = 4 * ti + 2 * m
                        bt = bbp.tile([128, N], F32, name="bt", tag="bt")
                        eng = bt_engs[ti]
                        eng.dma_start(
                            bt[0:64, :].unsqueeze(1),
                            inv_dram[rowA : rowA + 1, :].partition_broadcast(64),
                        )
                        eng.dma_start(
                            bt[64:128, :].unsqueeze(1),
                            inv_dram[rowA + 1 : rowA + 2, :].partition_broadcast(64),
                        )
                        if ti == 0:
                            nc.vector.scalar_tensor_tensor(
                                dsts[m][:],
                                dsts[m][:],
                                qsks8_sb[:],
                                bt[:],
                                OP.mult,
                                OP.mult,
                            )
                        else:
                            nc.vector.tensor_tensor(
                                dsts[m][:], dsts[m][:], bt[:], OP.mult
                            )

                for nm_ in range(NJ):
                    vproj_chunk(nm_)
                nc.sync.dma_start(
                    vext[:, :, :, D : D + 1],
                    vones_d[:].rearrange("p (j h) -> p j h", h=HPC).unsqueeze(3),
                )

            if dbg:
                for m in range(2):
                    nc.sync.dma_start(dbg_qn_d[m * 128 : (m + 1) * 128, :], qn[m][:])
                    nc.sync.dma_start(dbg_kn_d[m * 128 : (m + 1) * 128, :], kn[m][:])
                nc.sync.dma_start(
                    dbg_vext_d[:], vext[:].rearrange("p j h d -> p (j h d)")
                )

            # ---------------- phase 2: attention ----------------
            with (
                tc.tile_pool(name="simps", bufs=2, space="PSUM") as simps,
                tc.tile_pool(name="ops", bufs=2, space="PSUM") as ops,
                tc.tile_pool(name="at", bufs=4) as atp,
                tc.tile_pool(name="nrm", bufs=4) as nrm,
            ):
                for m in range(2):
                    hA, hB = 2 * m, 2 * m + 1
                    for qt in range(NQT):
                        qs_ = slice(qt * 512, (qt + 1) * 512)
                        oA = ops.tile([D + 1, 512], F32, name="oA", tag="oA")
                        oB = ops.tile([D + 1, 512], F32, name="oB", tag="oB")
                        for j in range(NJ):
                            js = slice(j * 128, (j + 1) * 128)
                            sim = simps.tile([128, 1024], F32, name="sim", tag="sim")
                            nc.tensor.matmul(
                                sim[:, 0:512],
                                lhsT=(kn[m][0:64, js]),
                                rhs=(qn[m][0:64, qs_]),
                                start=True,
                                stop=True,
                            )
                            nc.tensor.matmul(
                                sim[:, 512:1024],
                                lhsT=(kn[m][64:128, js]),
                                rhs=(qn[m][64:128, qs_]),
                                start=True,
                                stop=True,
                            )
                            at = atp.tile([128, 1024], BF16, name="at", tag="at")
                            nc.scalar.activation(at[:], sim[:], AF.Exp)
                            if dbg and m == 0 and qt == 0 and j == 0:
                                nc.sync.dma_start(dbg_at_d[:], at[:])
                            nc.tensor.matmul(
                                oA[:],
                                lhsT=(vext[:, j, hA, :]),
                                rhs=(at[:, 0:512]),
                                start=(j == 0),
                                stop=(j == NJ - 1),
                            )
                            nc.tensor.matmul(
                                oB[:],
                                lhsT=(vext[:, j, hB, :]),
                                rhs=(at[:, 512:1024]),
                                start=(j == 0),
                                stop=(j == NJ - 1),
                            )
                        for o_ps, base in ((oA, 0), (oB, 64)):
                            rsb = nrm.tile([1, 512], F32, name="rsb", tag="rsb")
                            nc.vector.tensor_copy(rsb[:], o_ps[D : D + 1, :])
                            br = nrm.tile([64, 512], F32, name="br", tag="br")
                            nc.gpsimd.partition_broadcast(br[:], rsb[:], channels=64)
                            bri = nrm.tile([64, 512], F32, name="bri", tag="bri")
                            nc.vector.reciprocal_approx_fast(bri[:], br[:])
                            nc.vector.tensor_tensor(
                                y[m][qt][base : base + 64, :],
                                o_ps[0:D, :],
                                bri[:],
                                OP.mult,
                            )

            if dbg:
                for m in range(2):
                    for qt in range(4):
                        nc.sync.dma_start(
                            dbg_y_d[
                                m * 128 : (m + 1) * 128, qt * 512 : (qt + 1) * 512
                            ],
                            y[m][qt][:],
                        )

            # ---------------- phase 3: output projection ----------------
            with (
                tc.tile_pool(name="wo", bufs=1) as wop,
                tc.tile_pool(name="fin", bufs=3) as finp,
                tc.tile_pool(name="pps", bufs=3, space="PSUM") as pps,
            ):
                wo_sb = []
                for m in range(2):
                    t = wop.tile([128, C], F32R, name=f"wo{m}", tag=f"wo{m}")
                    nc.sync.dma_start(t[:], woT_d[m * 128 : (m + 1) * 128, :])
                    wo_sb.append(t)
                out_engs = [nc.sync, nc.scalar, nc.gpsimd, nc.sync]
                for ct in range(NCT):
                    ot = finp.tile([128, N], F32, name="ot", tag=f"ot{ct}")
                    for qt in range(NQT):
                        qs_ = slice(qt * 512, (qt + 1) * 512)
                        pp = pps.tile([128, 512], F32, name="pp", tag="pp")
                        for m in range(2):
                            nc.tensor.matmul(
                                pp[:],
                                lhsT=(wo_sb[m][:, ct * 128 : (ct + 1) * 128]),
                                rhs=(y[m][qt][:]),
                                start=(m == 0),
                                stop=(m == 1),
                            )
                        nc.vector.tensor_scalar_add(
                            ot[:, qs_], pp[:], biasq_sb[:, ct : ct + 1]
                        )
                    out_engs[ct].dma_start(
                        out_d[ct * 128 : (ct + 1) * 128, :], ot[:]
                    )

    nc.finalize()
    return nc


def kernel(x, Wqkv, q_scale, k_scale, Wout, bout):
    global _CACHED_NC, LAST_RESULTS
    x = np.asarray(x, dtype=np.float32)
    Wqkv = np.asarray(Wqkv, dtype=np.float32)
    q_scale = np.asarray(q_scale, dtype=np.float32)
    k_scale = np.asarray(k_scale, dtype=np.float32)
    Wout = np.asarray(Wout, dtype=np.float32)
    bout = np.asarray(bout, dtype=np.float32)

    if _CACHED_NC is None:
        _CACHED_NC = build_nc()
    nc = _CACHED_NC

    H_DIM = HEADS * D
    qsks8 = np.tile((SCALE * q_scale * k_scale).astype(np.float32), 2)[:, None]
    qsks8 = np.ascontiguousarray(qsks8)
    biasq = np.ascontiguousarray((bout / 4.0).astype(np.float32)[:, None])
    onesw = np.zeros((128, 33), dtype=np.float32)
    onesw[0:64, 0] = 1.0
    onesw[64:128, 32] = 1.0
    import ml_dtypes
    vones = np.ones((128, 64), dtype=ml_dtypes.bfloat16)

    in_maps = []
    for core in range(NCORES):
        b = core // 4
        h0 = HPC * (core % 4)
        rs = slice(h0 * D, h0 * D + HPC * D)
        wq = Wqkv[0:H_DIM][rs]
        wk = Wqkv[H_DIM : 2 * H_DIM][rs]
        wv = Wqkv[2 * H_DIM : 3 * H_DIM][rs]
        in_maps.append(
            {
                "x": np.ascontiguousarray(x[b]),
                "wqT": np.ascontiguousarray(wq.T),
                "wkT": np.ascontiguousarray(wk.T),
                "wvT": np.ascontiguousarray(wv.T),
                "woT": np.ascontiguousarray(Wout[:, rs].T),
                "qsks8": qsks8,
                "onesw": onesw,
                "vones": vones,
                "biasq": biasq,
            }
        )

    res = run_bass_kernel_spmd(
        nc,
        in_maps,
        core_ids=list(range(NCORES)),
        trace=bool(os.environ.get("BASS_TRACE")),
        **EXTRA_RUN_KWARGS,
    )
    LAST_RESULTS = res

    outs = [np.asarray(res.results[i]["out"], dtype=np.float32) for i in range(NCORES)]
    full = np.empty((B, C, N), dtype=np.float32)
    full[0] = outs[0] + outs[1] + outs[2] + outs[3]
    full[1] = outs[4] + outs[5] + outs[6] + outs[7]
    return full



# revision 5
# speedup vs baseline: 1.1256x; 1.1256x over previous
"""Trainium2 Bass kernel for cosine-similarity ("sparse") attention.

Reference computation (B=2, C=512, N=2048, H=16, D=64, SCALE=8):
    qkv = Wqkv @ x                          # 1x1 conv
    q,k,v -> [B,H,D,N]
    q = l2norm(q, over D) * q_scale ; k = l2norm(k, over D) * k_scale
    sim = (q^T k) * 8 ; attn = softmax(sim, over keys)
    out = Wout @ (attn @ v) + bout

Sharding: 32 (batch, head) pairs across 8 cores -> each core owns one batch
(b = core//4) and 4 heads (h0 = 4*(core%4)).  Each core projects q/k/v for
its heads, runs attention, and computes a partial output projection
Wout[:, its-heads] @ y + bout/4.  Host sums the 4 partials per batch.

Device-side schedule (per core), tuned against the NTFF trace:
  - qT/kT kept as [D, N] so sim^T tiles [j, q] come straight off the PE;
    the two heads of a 128-partition pair use PE row-groups and stream
    back-to-back.  exp() is applied to simT on the ACT engine; attn @ v
    runs as oT[d, q] = v_ext[j, d].T @ attnT[j, q] accumulated over j,
    with a ones-column in v_ext producing the softmax denominator.
  - Softmax max-subtraction is skipped: sim = 8*cosine is bounded in [-8, 8].
  - Phase 2 is software-pipelined one j-step: the av pair for j-1 is
    emitted after the sim pair for j, so the PE's in-order queue never
    blocks on the current j's exp.  Steady state is ACT(exp)-bound at
    ~1.05us per j-step with the PE ~90% busy inside it.
  - l2norm uses a ones-indicator matmul for sumsq, then a fused
    PSUM->SBUF Rsqrt on the ACT engine (one activation table serves
    Square/Copy/Rsqrt, so phase 1 loads a single table).
  - Output projection for query-chunk qt is emitted right after the
    m=1 attention for qt, so only the last chunk's projection + DMA
    remain after the final av matmul.
"""

import os
import sys

import numpy as np

sys.path.insert(0, "/opt/trn_rl_repo")

import concourse.bass as bass  # noqa: E402
import concourse.mybir as mybir  # noqa: E402
from concourse import bacc, tile  # noqa: E402
from concourse.bass_utils import run_bass_kernel_spmd  # noqa: E402

F32 = mybir.dt.float32
F32R = mybir.dt.float32r
BF16 = mybir.dt.bfloat16
AF = mybir.ActivationFunctionType
OP = mybir.AluOpType

B, C, N = 2, 512, 2048
HEADS, D = 16, 64
SCALE = 8.0
NCORES = 8
HPC = 4  # heads per core

_CACHED_NC = None
LAST_RESULTS = None
EXTRA_RUN_KWARGS = {}


def build_nc():
    nc = bacc.Bacc(None, target_bir_lowering=False)

    x_d = nc.declare_dram_parameter("x", [C, N], F32R, isOutput=False)
    wqT_d = nc.declare_dram_parameter("wqT", [C, HPC * D], F32R, isOutput=False)
    wkT_d = nc.declare_dram_parameter("wkT", [C, HPC * D], F32R, isOutput=False)
    wvT_d = nc.declare_dram_parameter("wvT", [C, HPC * D], F32R, isOutput=False)
    woT_d = nc.declare_dram_parameter("woT", [HPC * D, C], F32R, isOutput=False)
    qsks8_d = nc.declare_dram_parameter("qsks8", [128, 1], F32, isOutput=False)
    onesw_d = nc.declare_dram_parameter("onesw", [128, 33], F32R, isOutput=False)
    vones_d = nc.declare_dram_parameter("vones", [128, 64], BF16, isOutput=False)
    biasq_d = nc.declare_dram_parameter("biasq", [C, 1], F32, isOutput=False)
    out_d = nc.declare_dram_parameter("out", [C, N], F32, isOutput=True)

    NQT = N // 512  # 4 query chunks of 512
    NJ = N // 128  # 16 key chunks of 128
    NCT = C // 128  # 4 channel chunks of 128

    with tile.TileContext(nc) as tc:
        with (
            tc.tile_pool(name="const", bufs=1) as const,
            tc.tile_pool(name="persist", bufs=1) as persist,
            tc.tile_pool(name="dramp", bufs=1, space="DRAM") as dramp,
        ):
            qsks8_sb = const.tile([128, 1], F32, name="qsks8", tag="qsks8")
            nc.sync.dma_start(qsks8_sb[:], qsks8_d[:])
            biasq_sb = const.tile([128, NCT], F32, name="biasq", tag="biasq")
            for ct in range(NCT):
                nc.sync.dma_start(
                    biasq_sb[:, ct : ct + 1], biasq_d[ct * 128 : (ct + 1) * 128, :]
                )
            # indicator weights: col 0 sums partitions 0-63 (head A), col 32
            # sums partitions 64-127 (head B); middle cols write zeros so the
            # [33, 512] sumsq psum rows land 32-aligned.  (DMA'd from host:
            # engines cannot memset float32r tiles.)
            ones_w = const.tile([128, 33], F32R, name="ones_w", tag="ones_w")
            nc.sync.dma_start(ones_w[:], onesw_d[:])
            wo_sb = []
            for m in range(2):
                t = const.tile([128, C], F32R, name=f"wo{m}", tag=f"wo{m}")
                nc.sync.dma_start(t[:], woT_d[m * 128 : (m + 1) * 128, :])
                wo_sb.append(t)

            # persistent tensors
            qn = [persist.tile([128, N], F32R, name=f"qn{m}", tag=f"qn{m}") for m in range(2)]
            kn = [persist.tile([128, N], F32R, name=f"kn{m}", tag=f"kn{m}") for m in range(2)]
            y = [
                [
                    persist.tile([128, 512], F32R, name=f"y{m}_{qt}", tag=f"y{m}_{qt}")
                    for qt in range(4)
                ]
                for m in range(2)
            ]
            vext = persist.tile([128, NJ, HPC, D + 1], BF16, name="vext", tag="vext")
            inv_dram = dramp.tile([8, N], F32, name="inv_dram", tag="inv_dram")
            # softmax-denominator ones column (disjoint bytes from the vproj
            # copies; issued up front so it is never waited on)
            nc.sync.dma_start(
                vext[:, :, :, D : D + 1],
                vones_d[:].rearrange("p (j h) -> p j h", h=HPC).unsqueeze(3),
            )

            # ---------------- phase 1: projections + norms ----------------
            with (
                tc.tile_pool(name="xw", bufs=1) as xw,
                tc.tile_pool(name="raw", bufs=1) as rawp,
                tc.tile_pool(name="sq", bufs=3) as sqp,
                tc.tile_pool(name="bb", bufs=2) as bbp,
                tc.tile_pool(name="prps", bufs=3, space="PSUM") as prps,
                tc.tile_pool(name="ssps", bufs=2, space="PSUM") as ssps,
            ):
                # sqrt(sumsq) tiles; head A at row 0 / head B at row 32,
                # filled chunk-by-chunk by fused PSUM->SBUF Sqrt on ACT
                # (Rsqrt is rejected by bass for accuracy), then inverted
                # whole by one DVE reciprocal per (t, m)
                srt_tm = [
                    [
                        rawp.tile([33, N], F32, name=f"srt{t}{m}", tag=f"srt{t}{m}")
                        for m in range(2)
                    ]
                    for t in range(2)
                ]
                inv_tm = [
                    [
                        rawp.tile([33, N], F32, name=f"inv{t}{m}", tag=f"inv{t}{m}")
                        for m in range(2)
                    ]
                    for t in range(2)
                ]
                # DMA order tuned for earliest first matmul: wq, x[nt0], wk,
                # x[nt1..3], wv; triggers spread over the DMA-capable queues
                wq_all = xw.tile([128, NCT, HPC * D], F32R, name="wq_all", tag="wq_all")
                nc.scalar.dma_start(
                    wq_all[:], wqT_d[:].rearrange("(c p) d -> p c d", p=128)
                )
                wq_sb = [wq_all[:, c, :] for c in range(NCT)]
                dma_engs = [nc.sync, nc.scalar, nc.gpsimd, nc.sync]
                x_sb = [[None] * NQT for _ in range(NCT)]
                for c in range(NCT):
                    t = xw.tile([128, 512], F32R, name=f"x{c}_0", tag=f"x{c}_0")
                    dma_engs[c].dma_start(t[:], x_d[c * 128 : (c + 1) * 128, 0:512])
                    x_sb[c][0] = t
                wk_all = xw.tile([128, NCT, HPC * D], F32R, name="wk_all", tag="wk_all")
                nc.gpsimd.dma_start(
                    wk_all[:], wkT_d[:].rearrange("(c p) d -> p c d", p=128)
                )
                wk_sb = [wk_all[:, c, :] for c in range(NCT)]
                wv_all = xw.tile([128, NCT, HPC * D], F32R, name="wv_all", tag="wv_all")
                nc.scalar.dma_start(
                    wv_all[:], wvT_d[:].rearrange("(c p) d -> p c d", p=128)
                )
                wv_sb = [wv_all[:, c, :] for c in range(NCT)]
                for nt in range(1, NQT):
                    for c in range(NCT):
                        t = xw.tile([128, 512], F32R, name=f"x{c}_{nt}", tag=f"x{c}_{nt}")
                        dma_engs[c].dma_start(
                            t[:], x_d[c * 128 : (c + 1) * 128, nt * 512 : (nt + 1) * 512]
                        )
                        x_sb[c][nt] = t

                # sumsq matmuls are emitted one proj-chunk late so the PE
                # never waits on the ACT square of the current chunk
                pend_ss = []

                def emit_ss(limit):
                    while len(pend_ss) > limit:
                        sq_t, ti_, m_, nt_ = pend_ss.pop(0)
                        ss = ssps.tile([33, 512], F32, name="ss", tag="ss")
                        nc.tensor.matmul(
                            ss[:], lhsT=(ones_w[:]), rhs=(sq_t[:]), start=True, stop=True
                        )
                        nc.scalar.activation(
                            srt_tm[ti_][m_][:, nt_ * 512 : (nt_ + 1) * 512],
                            ss[:],
                            AF.Sqrt,
                        )

                def proj_chunk(m, w_sb, raws, ti, nt):
                    ps = prps.tile([128, 512], F32, name="pr", tag="pr")
                    for c in range(NCT):
                        nc.tensor.matmul(
                            ps[:],
                            lhsT=(w_sb[c][:, m * 128 : (m + 1) * 128]),
                            rhs=(x_sb[c][nt][:]),
                            start=(c == 0),
                            stop=(c == NCT - 1),
                        )
                    emit_ss(1)
                    nc.vector.tensor_copy(
                        raws[m][:, nt * 512 : (nt + 1) * 512], ps[:]
                    )
                    sq = sqp.tile([128, 512], F32R, name="sq", tag="sq")
                    nc.scalar.activation(sq[:], ps[:], AF.Square)
                    pend_ss.append((sq, ti, m, nt))

                for m in range(2):
                    for nt in range(NQT):
                        proj_chunk(m, wq_sb, qn, 0, nt)
                        proj_chunk(m, wk_sb, kn, 1, nt)
                    emit_ss(0)
                    for ti in range(2):
                        nc.vector.reciprocal_approx_fast(
                            inv_tm[ti][m][:], srt_tm[ti][m][:]
                        )
                    # norm chain for pair m: DMA the inverse-norm rows out,
                    # broadcast them back over 64 partitions, then scale
                    # qn (DVE, with the folded 8*qs*ks) and kn (GpSimd)
                    for ti in range(2):
                        nc.sync.dma_start(
                            inv_dram[4 * ti + 2 * m : 4 * ti + 2 * m + 2, :],
                            inv_tm[ti][m][0:33:32, :],
                        )
                    bt_engs = [nc.sync, nc.gpsimd]
                    for dsts, ti in ((qn, 0), (kn, 1)):
                        rowA = 4 * ti + 2 * m
                        bt = bbp.tile([128, N], F32, name="bt", tag="bt")
                        eng = bt_engs[ti]
                        eng.dma_start(
                            bt[0:64, :].unsqueeze(1),
                            inv_dram[rowA : rowA + 1, :].partition_broadcast(64),
                        )
                        eng.dma_start(
                            bt[64:128, :].unsqueeze(1),
                            inv_dram[rowA + 1 : rowA + 2, :].partition_broadcast(64),
                        )
                        if ti == 0:
                            nc.vector.scalar_tensor_tensor(
                                dsts[m][:],
                                dsts[m][:],
                                qsks8_sb[:],
                                bt[:],
                                OP.mult,
                                OP.mult,
                            )
                        else:
                            nc.gpsimd.tensor_tensor(
                                dsts[m][:], dsts[m][:], bt[:], OP.mult
                            )

                # v projection; vext copies ride the DVE (ACT keeps the
                # Rsqrt table warm until here, Exp table loads once after)
                for nm_ in range(NJ):
                    psv = prps.tile([128, HPC * D], F32, name="prv", tag="pr")
                    for c in range(NCT):
                        nc.tensor.matmul(
                            psv[:],
                            lhsT=(
                                x_sb[c][nm_ // 4][
                                    :, (nm_ % 4) * 128 : (nm_ % 4) * 128 + 128
                                ]
                            ),
                            rhs=(wv_sb[c][:]),
                            start=(c == 0),
                            stop=(c == NCT - 1),
                        )
                    nc.vector.tensor_copy(
                        vext[:, nm_, :, 0:D],
                        psv[:].rearrange("p (h d) -> p h d", h=HPC),
                    )

            # ---------------- phase 2: attention + fused out-proj ----------
            with (
                tc.tile_pool(name="simps", bufs=2, space="PSUM") as simps,
                tc.tile_pool(name="accps", bufs=4, space="PSUM") as accps,
                tc.tile_pool(name="at", bufs=4) as atp,
                tc.tile_pool(name="nrm", bufs=4) as nrm,
                tc.tile_pool(name="fin", bufs=4) as finp,
            ):
                out_engs = [nc.sync, nc.gpsimd, nc.sync, nc.gpsimd]

                def out_proj(qt):
                    qs_ = slice(qt * 512, (qt + 1) * 512)
                    for ct in range(NCT):
                        pp = accps.tile([128, 512], F32, name="pp", tag="acc")
                        for m in range(2):
                            nc.tensor.matmul(
                                pp[:],
                                lhsT=(wo_sb[m][:, ct * 128 : (ct + 1) * 128]),
                                rhs=(y[m][qt][:]),
                                start=(m == 0),
                                stop=(m == 1),
                            )
                        ot = finp.tile([128, 512], F32, name="ot", tag="ot")
                        nc.vector.tensor_scalar_add(
                            ot[:], pp[:], biasq_sb[:, ct : ct + 1]
                        )
                        out_engs[ct].dma_start(
                            out_d[ct * 128 : (ct + 1) * 128, qs_], ot[:]
                        )

                for m in range(2):
                    hA, hB = 2 * m, 2 * m + 1
                    for qt in range(NQT):
                        qs_ = slice(qt * 512, (qt + 1) * 512)
                        oA = accps.tile([D + 1, 512], F32, name="oA", tag="acc")
                        oB = accps.tile([D + 1, 512], F32, name="oB", tag="acc")

                        def av_pair(at_t, j_):
                            nc.tensor.matmul(
                                oA[:],
                                lhsT=(vext[:, j_, hA, :]),
                                rhs=(at_t[:, 0:512]),
                                start=(j_ == 0),
                                stop=(j_ == NJ - 1),
                            )
                            nc.tensor.matmul(
                                oB[:],
                                lhsT=(vext[:, j_, hB, :]),
                                rhs=(at_t[:, 512:1024]),
                                start=(j_ == 0),
                                stop=(j_ == NJ - 1),
                            )

                        pend_av = None
                        for j in range(NJ):
                            js = slice(j * 128, (j + 1) * 128)
                            sim = simps.tile([128, 1024], F32, name="sim", tag="sim")
                            nc.tensor.matmul(
                                sim[:, 0:512],
                                lhsT=(kn[m][0:64, js]),
                                rhs=(qn[m][0:64, qs_]),
                                start=True,
                                stop=True,
                            )
                            nc.tensor.matmul(
                                sim[:, 512:1024],
                                lhsT=(kn[m][64:128, js]),
                                rhs=(qn[m][64:128, qs_]),
                                start=True,
                                stop=True,
                            )
                            if pend_av is not None:
                                av_pair(*pend_av)
                            at = atp.tile([128, 1024], BF16, name="at", tag="at")
                            nc.scalar.activation(at[:], sim[:], AF.Exp)
                            pend_av = (at, j)
                        av_pair(*pend_av)

                        for o_ps, base in ((oA, 0), (oB, 64)):
                            rsb = nrm.tile([1, 512], F32, name="rsb", tag="rsb")
                            nc.vector.tensor_copy(rsb[:], o_ps[D : D + 1, :])
                            br = nrm.tile([64, 512], F32, name="br", tag="br")
                            nc.gpsimd.partition_broadcast(br[:], rsb[:], channels=64)
                            bri = nrm.tile([64, 512], F32, name="bri", tag="bri")
                            nc.vector.reciprocal_approx_fast(bri[:], br[:])
                            nc.vector.tensor_tensor(
                                y[m][qt][base : base + 64, :],
                                o_ps[0:D, :],
                                bri[:],
                                OP.mult,
                            )
                        if m == 1:
                            out_proj(qt)

    nc.finalize()
    return nc


def kernel(x, Wqkv, q_scale, k_scale, Wout, bout):
    global _CACHED_NC, LAST_RESULTS
    x = np.asarray(x, dtype=np.float32)
    Wqkv = np.asarray(Wqkv, dtype=np.float32)
    q_scale = np.asarray(q_scale, dtype=np.float32)
    k_scale = np.asarray(k_scale, dtype=np.float32)
    Wout = np.asarray(Wout, dtype=np.float32)
    bout = np.asarray(bout, dtype=np.float32)

    if _CACHED_NC is None:
        _CACHED_NC = build_nc()
    nc = _CACHED_NC

    H_DIM = HEADS * D
    qsks8 = np.tile((SCALE * q_scale * k_scale).astype(np.float32), 2)[:, None]
    qsks8 = np.ascontiguousarray(qsks8)
    biasq = np.ascontiguousarray((bout / 4.0).astype(np.float32)[:, None])
    onesw = np.zeros((128, 33), dtype=np.float32)
    onesw[0:64, 0] = 1.0
    onesw[64:128, 32] = 1.0
    import ml_dtypes
    vones = np.ones((128, 64), dtype=ml_dtypes.bfloat16)

    in_maps = []
    for core in range(NCORES):
        b = core // 4
        h0 = HPC * (core % 4)
        rs = slice(h0 * D, h0 * D + HPC * D)
        wq = Wqkv[0:H_DIM][rs]
        wk = Wqkv[H_DIM : 2 * H_DIM][rs]
        wv = Wqkv[2 * H_DIM : 3 * H_DIM][rs]
        in_maps.append(
            {
                "x": np.ascontiguousarray(x[b]),
                "wqT": np.ascontiguousarray(wq.T),
                "wkT": np.ascontiguousarray(wk.T),
                "wvT": np.ascontiguousarray(wv.T),
                "woT": np.ascontiguousarray(Wout[:, rs].T),
                "qsks8": qsks8,
                "onesw": onesw,
                "vones": vones,
                "biasq": biasq,
            }
        )

    res = run_bass_kernel_spmd(
        nc,
        in_maps,
        core_ids=list(range(NCORES)),
        trace=bool(os.environ.get("BASS_TRACE")),
        **EXTRA_RUN_KWARGS,
    )
    LAST_RESULTS = res

    outs = [np.asarray(res.results[i]["out"], dtype=np.float32) for i in range(NCORES)]
    full = np.empty((B, C, N), dtype=np.float32)
    full[0] = outs[0] + outs[1] + outs[2] + outs[3]
    full[1] = outs[4] + outs[5] + outs[6] + outs[7]
    return full
